# revision 12
# baseline (speedup 1.0000x reference)
"""DHGNN baseline kernel for 8 Trainium2 NeuronCores.

Sharding (graph/data parallel):
 - node GRU: each core encodes its own node chunk (50000/8, padded to 7168).
 - edge GRU + decoder: each core processes a contiguous 100000-edge chunk.
 - GCN: edges assigned to the core owning their dst node; per-edge source
   features come from a replicated (AllGather'd) bf16 node table via
   dma_gather; the segment-sum over dst uses one-hot selection matmuls over
   dst-sorted edges accumulating in PSUM.
The host only shards, permutes memory layouts, and builds integer index
metadata; all floating-point math runs on device.
"""

import math
import numpy as np

import concourse.bacc as bacc
import concourse.bass as bass
import concourse.mybir as mybir
import concourse.tile as tile
from concourse import bass_utils
from concourse.masks import make_identity

F32 = mybir.dt.float32
F32R = mybir.dt.float32r
BF16 = mybir.dt.bfloat16
I16 = mybir.dt.int16
I32 = mybir.dt.int32

HALF = 32768          # int16 split point for gather indices


class Cfg:
    def __init__(self, N, E, T, IN_DIM, EDGE_DIM, HID, n_cores):
        self.N, self.E, self.T = N, E, T
        self.IN, self.ED, self.HID = IN_DIM, EDGE_DIM, HID
        self.C = n_cores
        assert N % n_cores == 0 and E % n_cores == 0
        self.n_per = N // n_cores
        self.node_st = 512
        self.n_stn = math.ceil(self.n_per / self.node_st)
        self.n_pad = self.n_stn * self.node_st
        self.nt128 = self.n_pad // 128
        self.Npad = self.n_pad * n_cores
        self.e_per = E // n_cores
        self.edge_st = 2048
        self.n_ste = math.ceil(self.e_per / self.edge_st)
        self.e_pad = self.n_ste * self.edge_st
        self.jg = 4                      # node tiles per GCN gather group


def _kron8(W):
    return np.kron(np.eye(8, dtype=np.float32), np.ascontiguousarray(W.T))


# ----------------------------------------------------------------------------
# host-side prep
# ----------------------------------------------------------------------------

def prep_inputs(cfg, inputs):
    N, E, T, C = cfg.N, cfg.E, cfg.T, cfg.C
    IN, ED = cfg.IN, cfg.ED
    node_seq = np.asarray(inputs["node_seq"], np.float32)
    edge_seq = np.asarray(inputs["edge_seq"], np.float32)
    edge_index = np.asarray(inputs["edge_index"])
    src = edge_index[0].astype(np.int64)
    dst = edge_index[1].astype(np.int64)
    deg = (np.bincount(dst, minlength=N) + 1).astype(np.float32)

    per_core = [dict() for _ in range(C)]
    meta = {}

    # ---- node GRU data [n_stn, IN, T, node_st] ----
    for k in range(C):
        pad = np.zeros((cfg.n_pad, T, IN), np.float32)
        pad[:cfg.n_per] = node_seq[k * cfg.n_per:(k + 1) * cfg.n_per]
        a = pad.reshape(cfg.n_stn, cfg.node_st, T, IN).transpose(0, 3, 2, 1)
        per_core[k]["nd_gru"] = np.ascontiguousarray(a)
        d = np.ones(cfg.n_pad, np.float32)
        d[:cfg.n_per] = deg[k * cfg.n_per:(k + 1) * cfg.n_per]
        per_core[k]["deg_T"] = np.ascontiguousarray(
            d.reshape(cfg.nt128, 128).T)

    # ---- edge GRU data [n_ste, 8*ED, T, 256] ----
    for k in range(C):
        pad = np.zeros((cfg.e_pad, T, ED), np.float32)
        pad[:cfg.e_per] = edge_seq[k * cfg.e_per:(k + 1) * cfg.e_per]
        a = pad.reshape(cfg.n_ste, 8, 256, T, ED).transpose(0, 1, 4, 3, 2)
        per_core[k]["ed_gru"] = np.ascontiguousarray(
            a.reshape(cfg.n_ste, 8 * ED, T, 256))

    # ---- GCN: per-core dst-owned edges, dst-sorted, L/H split on padded src
    owner = dst // cfg.n_per
    pg = (src // cfg.n_per) * cfg.n_pad + (src % cfg.n_per)
    core_edges = []
    for k in range(C):
        sel = np.nonzero(owner == k)[0]
        dl = dst[sel] - k * cfg.n_per
        sg = pg[sel]
        hi = (sg >= HALF).astype(np.int64)
        order = np.lexsort((dl, hi))
        core_edges.append((dl[order], sg[order], int((hi == 0).sum())))
    ntL = max(math.ceil(m[2] / 128) for m in core_edges)
    ntH = max(math.ceil((len(m[0]) - m[2]) / 128) for m in core_edges)
    NT = ntL + ntH
    meta["ntL"], meta["ntH"], meta["NT"] = ntL, ntH, NT

    # per-core padded tile arrays + union pair schedule
    all_dl = []
    for k in range(C):
        dl, sg, nL = core_edges[k]
        nH = len(dl) - nL
        dla = np.full(NT * 128, -1, np.int64)
        sga = np.zeros(NT * 128, np.int64)
        dla[:nL], sga[:nL] = dl[:nL], sg[:nL]
        dla[ntL * 128:ntL * 128 + nH] = dl[nL:]
        sga[ntL * 128:ntL * 128 + nH] = sg[nL:]
        gl = np.where(dla[:ntL * 128] >= 0, sga[:ntL * 128], 0)
        gh = np.where(dla[ntL * 128:] >= 0, sga[ntL * 128:] - HALF, 0)
        per_core[k]["idxL"] = np.ascontiguousarray(
            np.tile(gl.reshape(-1, 16).T.astype(np.int16), (8, 1)))
        ihw = gh.reshape(-1, 16).T.astype(np.int16) if gh.size else \
            np.zeros((16, 8), np.int16)
        per_core[k]["idxH"] = np.ascontiguousarray(np.tile(ihw, (8, 1)))
        all_dl.append(dla)

    pair_keys = set()
    core_pairs = []
    for k in range(C):
        dla = all_dl[k]
        pd = {}
        for m in range(NT):
            d128 = dla[m * 128:(m + 1) * 128]
            real = d128 >= 0
            if not real.any():
                continue
            for t in range(int(d128[real].min() // 128),
                           int(d128[real].max() // 128) + 1):
                offs = np.where(real, d128 - t * 128, -4096).astype(np.float32)
                pd[(t, m)] = offs
                pair_keys.add((t, m))
        core_pairs.append(pd)
    sched = sorted(pair_keys)                      # grouped by node tile t
    NPs = max(len(sched), 1)
    meta["sched"] = sched
    meta["NPs"] = NPs
    for k in range(C):
        offs_T = np.full((128, NPs), -4096.0, np.float32)
        for q, key in enumerate(sched):
            if key in core_pairs[k]:
                offs_T[:, q] = core_pairs[k][key]
        per_core[k]["offs_T"] = np.ascontiguousarray(offs_T)

    # gather groups: node tiles grouped by cfg.jg; m-ranges per section
    groups = []
    n_groups = math.ceil(cfg.nt128 / cfg.jg)
    for gidx in range(n_groups):
        j0, j1 = gidx * cfg.jg, min((gidx + 1) * cfg.jg, cfg.nt128)
        qs = [q for q, (t, m) in enumerate(sched) if j0 <= t < j1]
        msL = [sched[q][1] for q in qs if sched[q][1] < ntL]
        msH = [sched[q][1] for q in qs if sched[q][1] >= ntL]
        mL = (min(msL), max(msL) + 1) if msL else (0, 0)
        mH = (min(msH), max(msH) + 1) if msH else (ntL, ntL)
        groups.append((j0, j1, mL, mH, qs))
    meta["groups"] = groups

    # ---- weights ----
    w = {n: np.asarray(inputs[n], np.float32) for n in
         ["W_ih_n", "W_hh_n", "b_ih_n", "b_hh_n",
          "W_ih_e", "W_hh_e", "b_ih_e", "b_hh_e",
          "W1", "b1", "W2", "b2", "Wf", "bf"]}
    H = ED
    eK = np.concatenate([
        _kron8(w["W_ih_e"][0:H]), _kron8(w["W_hh_e"][0:H]),
        _kron8(w["W_ih_e"][H:2 * H]), _kron8(w["W_hh_e"][H:2 * H]),
        _kron8(w["W_ih_e"][2 * H:3 * H]), _kron8(w["W_hh_e"][2 * H:3 * H]),
    ], axis=1)                                     # [128, 768]
    ebias = np.zeros((1, 512), np.float32)
    for g in range(8):
        ebias[0, 0 + g * H:0 + (g + 1) * H] = (w["b_ih_e"][0:H]
                                               + w["b_hh_e"][0:H])
        ebias[0, 128 + g * H:128 + (g + 1) * H] = (w["b_ih_e"][H:2 * H]
                                                   + w["b_hh_e"][H:2 * H])
        ebias[0, 256 + g * H:256 + (g + 1) * H] = w["b_ih_e"][2 * H:3 * H]
        ebias[0, 384 + g * H:384 + (g + 1) * H] = w["b_hh_e"][2 * H:3 * H]
    D = IN
    nK = np.concatenate([
        w["W_ih_n"][0:D].T, w["W_hh_n"][0:D].T,
        w["W_ih_n"][D:2 * D].T, w["W_hh_n"][D:2 * D].T,
        w["W_ih_n"][2 * D:3 * D].T, w["W_hh_n"][2 * D:3 * D].T,
    ], axis=1)                                     # [64, 384]
    nbias = np.zeros((1, 256), np.float32)
    nbias[0, 0:D] = w["b_ih_n"][0:D] + w["b_hh_n"][0:D]
    nbias[0, 64:64 + D] = w["b_ih_n"][D:2 * D] + w["b_hh_n"][D:2 * D]
    nbias[0, 128:128 + D] = w["b_ih_n"][2 * D:3 * D]
    nbias[0, 192:192 + D] = w["b_hh_n"][2 * D:3 * D]
    BDf = _kron8(w["Wf"])                          # [128, 128]
    bfp = np.zeros((1, 128), np.float32)
    for g in range(8):
        bfp[0, g * H:(g + 1) * H] = w["bf"]
    wk = dict(eK=np.ascontiguousarray(eK.astype(np.float32)),
              e_bias=ebias, nK=np.ascontiguousarray(nK.astype(np.float32)),
              n_bias=nbias, BDf=np.ascontiguousarray(BDf.astype(np.float32)),
              bf_p=bfp,
              W1=np.ascontiguousarray(w["W1"]),
              W2=np.ascontiguousarray(w["W2"]),
              b1=w["b1"][None, :].copy(), b2=w["b2"][None, :].copy())
    meta["e_bias_nz"] = bool(np.any(ebias != 0))
    meta["n_bias_nz"] = bool(np.any(nbias != 0))
    meta["bf_nz"] = bool(np.any(w["bf"] != 0))
    meta["b1_nz"] = bool(np.any(w["b1"] != 0))
    meta["b2_nz"] = bool(np.any(w["b2"] != 0))
    for k in range(C):
        per_core[k].update(wk)
    return per_core, meta


# ----------------------------------------------------------------------------
# device program
# ----------------------------------------------------------------------------

def build_program(cfg, meta):
    nc = bacc.Bacc("TRN2", num_devices=cfg.C, debug=False)
    T, IN, ED, HID = cfg.T, cfg.IN, cfg.ED, cfg.HID
    nt128 = cfg.nt128
    ntL, ntH, NT = meta["ntL"], meta["ntH"], meta["NT"]
    NPs, sched, groups = meta["NPs"], meta["sched"], meta["groups"]
    Sig = mybir.ActivationFunctionType.Sigmoid
    Tnh = mybir.ActivationFunctionType.Tanh
    Relu = mybir.ActivationFunctionType.Relu
    mult, addop = mybir.AluOpType.mult, mybir.AluOpType.add
    sub, iseq = mybir.AluOpType.subtract, mybir.AluOpType.is_equal

    USE_F32R = False

    def r(ap):
        return ap.bitcast(F32R) if USE_F32R else ap

    def mm_chain(out, steps):
        """steps: list of (lhsT, rhs); emits an accumulation group."""
        for i, (lt, rh) in enumerate(steps):
            nc.tensor.matmul(out=out, lhsT=lt, rhs=rh, start=(i == 0),
                             stop=(i == len(steps) - 1),
                             skip_group_check=(len(steps) > 1))

    din = {}
    for name, shape, dt in [
        ("nd_gru", [cfg.n_stn, IN, T, cfg.node_st], F32),
        ("ed_gru", [cfg.n_ste, 8 * ED, T, 256], F32),
        ("deg_T", [128, nt128], F32),
        ("idxL", [128, ntL * 8], I16),
        ("idxH", [128, max(ntH, 1) * 8], I16),
        ("offs_T", [128, NPs], F32),
        ("eK", [128, 768], F32), ("e_bias", [1, 512], F32),
        ("nK", [64, 384], F32), ("n_bias", [1, 256], F32),
        ("BDf", [128, 128], F32), ("bf_p", [1, 128], F32),
        ("W1", [IN, HID], F32), ("W2", [HID, IN], F32),
        ("b1", [1, HID], F32), ("b2", [1, IN], F32),
    ]:
        din[name] = nc.dram_tensor(name, shape, dt, kind="ExternalInput").ap()

    x_rec = nc.dram_tensor("x_rec", [cfg.n_pad, IN], F32,
                           kind="ExternalOutput").ap()
    e_rec = nc.dram_tensor("e_rec", [cfg.e_pad, ED], F32,
                           kind="ExternalOutput").ap()
    RG = [[i for i in range(cfg.C)]]

    with tile.TileContext(nc) as tc:
        with (
            tc.tile_pool(name="const", bufs=1) as cpool,
            tc.tile_pool(name="persist", bufs=1) as ppool,
            tc.tile_pool(name="dram", bufs=1, space="DRAM") as dpool,
        ):
            sb = {}
            for name in ["eK", "e_bias", "nK", "n_bias", "BDf", "bf_p",
                         "W1", "W2", "b1", "b2", "deg_T", "offs_T"]:
                t_ = cpool.tile(din[name].shape, F32, name="c_" + name)
                nc.sync.dma_start(out=t_[:], in_=din[name])
                sb[name] = t_
            ones512 = cpool.tile([1, 512], F32)
            nc.gpsimd.memset(ones512[:], 1.0)
            iota_i32 = cpool.tile([128, 128], I32)
            nc.gpsimd.iota(iota_i32[:], pattern=[[1, 128]], base=0,
                           channel_multiplier=0)
            iota_bf = cpool.tile([128, 128], BF16)
            nc.vector.tensor_copy(out=iota_bf[:], in_=iota_i32[:])
            ident64 = cpool.tile([64, 64], F32)
            make_identity(nc, ident64[:])
            ident128 = cpool.tile([128, 128], F32)
            make_identity(nc, ident128[:])

            # dinv = 1/sqrt(deg), with one Newton step to clean up ACT sqrt:
            # y = sqrt(x); y1 = 0.5*(y + x/y)
            rec = ppool.tile([128, nt128], F32)
            nc.vector.reciprocal(out=rec[:], in_=sb["deg_T"][:])   # x = 1/deg
            y0 = ppool.tile([128, nt128], F32)
            nc.scalar.sqrt(out=y0[:], in_=rec[:])
            yr = ppool.tile([128, nt128], F32)
            nc.vector.reciprocal(out=yr[:], in_=y0[:])
            dinv = ppool.tile([128, nt128], F32)
            nc.vector.tensor_tensor(out=dinv[:], in0=rec[:], in1=yr[:],
                                    op=mult)                       # x/y
            nc.vector.tensor_tensor(out=dinv[:], in0=dinv[:], in1=y0[:],
                                    op=addop)
            nc.vector.tensor_scalar(out=dinv[:], in0=dinv[:], scalar1=0.5,
                                    scalar2=None, op0=mult)
            dinv2 = ppool.tile([128, nt128], F32)
            nc.vector.tensor_tensor(out=dinv2[:], in0=dinv[:], in1=dinv[:],
                                    op=mult)

            x_own = ppool.tile([128, nt128 * IN], F32)
            h_own = ppool.tile([128, nt128 * HID], F32)

            xtab_own = dpool.tile([cfg.n_pad, 128], BF16)
            xtab = dpool.tile([cfg.Npad, 128], BF16, addr_space="Shared")
            htab_own = dpool.tile([cfg.n_pad, 128], BF16)
            htab = dpool.tile([cfg.Npad, 128], BF16, addr_space="Shared")

            nKs, ebs, nbs = sb["nK"], sb["e_bias"], sb["n_bias"]

            # ============ Phase 1: node GRU ============
            # state/gates feature-major [64, 512] per 512-node supertile
            with (
                tc.tile_pool(name="ngru", bufs=2) as npool,
                tc.tile_pool(name="ngru_ps", bufs=1, space="PSUM") as nps,
                tc.tile_pool(name="ngru_ps2", bufs=2, space="PSUM") as nps2,
                tc.tile_pool(name="ngru_st", bufs=1) as nst,
            ):
                for st in range(cfg.n_stn):
                    xt_all = npool.tile([64, T * 512], F32, tag="xt")
                    nc.sync.dma_start(
                        out=xt_all[:],
                        in_=din["nd_gru"][st].rearrange("f t c -> f (t c)"))
                    hT = nst.tile([64, 512], F32, tag="hT")
                    for t in range(T):
                        xx = xt_all[:, t * 512:(t + 1) * 512]
                        ps_rz = nps.tile([64, 1024], F32, tag="psrz")
                        ps_gin = nps.tile([64, 512], F32, tag="psgin")
                        for gate, col, w0, b0 in (("r", 0, 0, 0),
                                                  ("z", 512, 128, 64)):
                            o = ps_rz[:, col:col + 512]
                            steps = [(r(nKs[:, w0:w0 + 64]), r(xx))]
                            if t > 0:
                                steps.append((r(nKs[:, w0 + 64:w0 + 128]),
                                              r(hT[:])))
                            if meta["n_bias_nz"]:
                                steps.append((r(nbs[0:1, b0:b0 + 64]),
                                              r(ones512[:, 0:512])))
                            mm_chain(o, steps)
                        steps = [(r(nKs[:, 256:320]), r(xx))]
                        if meta["n_bias_nz"]:
                            steps.append((r(nbs[0:1, 128:192]),
                                          r(ones512[:, 0:512])))
                        mm_chain(ps_gin[:], steps)
                        if t > 0:
                            ps_ghn = nps.tile([64, 512], F32, tag="psghn")
                            steps = [(r(nKs[:, 320:384]), r(hT[:]))]
                            if meta["n_bias_nz"]:
                                steps.append((r(nbs[0:1, 192:256]),
                                              r(ones512[:, 0:512])))
                            mm_chain(ps_ghn[:], steps)
                        rz = npool.tile([64, 1024], F32, tag="rz")
                        nc.scalar.activation(out=rz[:], in_=ps_rz[:], func=Sig)
                        n_sb = npool.tile([64, 512], F32, tag="nsb")
                        if t > 0:
                            m1 = npool.tile([64, 512], F32, tag="m1")
                            nc.vector.tensor_tensor(out=m1[:],
                                                    in0=rz[:, 0:512],
                                                    in1=ps_ghn[:], op=mult)
                            nc.vector.tensor_tensor(out=m1[:], in0=m1[:],
                                                    in1=ps_gin[:], op=addop)
                            nc.scalar.activation(out=n_sb[:], in_=m1[:],
                                                 func=Tnh)
                            d = npool.tile([64, 512], F32, tag="d")
                            nc.gpsimd.tensor_tensor(out=d[:], in0=hT[:],
                                                    in1=n_sb[:], op=sub)
                            nc.gpsimd.tensor_tensor(out=d[:],
                                                    in0=rz[:, 512:1024],
                                                    in1=d[:], op=mult)
                            nc.vector.tensor_tensor(out=hT[:], in0=n_sb[:],
                                                    in1=d[:], op=addop)
                        else:
                            nc.scalar.activation(out=n_sb[:], in_=ps_gin[:],
                                                 func=Tnh)
                            zn = npool.tile([64, 512], F32, tag="d")
                            nc.gpsimd.tensor_tensor(out=zn[:],
                                                    in0=rz[:, 512:1024],
                                                    in1=n_sb[:], op=mult)
                            nc.vector.tensor_tensor(out=hT[:], in0=n_sb[:],
                                                    in1=zn[:], op=sub)
                    # x_t node-major + bf16 table rows (4 tiles of 128 nodes)
                    for j8 in range(4):
                        j = st * 4 + j8
                        pxt = nps2.tile([128, 64], F32, tag="pxt")
                        nc.tensor.transpose(
                            out=r(pxt[:]),
                            in_=r(hT[:, j8 * 128:(j8 + 1) * 128]),
                            identity=r(ident64[:]))
                        nc.vector.tensor_copy(
                            out=x_own[:, j * IN:(j + 1) * IN], in_=pxt[:])
                        xs = npool.tile([128, 128], BF16, tag="xs")
                        nc.gpsimd.memset(xs[:], 0.0)
                        nc.vector.tensor_scalar(
                            out=xs[:, 0:IN],
                            in0=x_own[:, j * IN:(j + 1) * IN],
                            scalar1=dinv[:, j:j + 1], scalar2=None, op0=mult)
                        nc.sync.dma_start(
                            out=xtab_own[j * 128:(j + 1) * 128, :], in_=xs[:])

            nc.gpsimd.collective_compute(
                "AllGather", mybir.AluOpType.bypass, replica_groups=RG,
                ins=[xtab_own[:, :]], outs=[xtab[:, :]])

            # ============ Phase 2: edge GRU + decode ============
            eKs = sb["eK"]
            with (
                tc.tile_pool(name="egru", bufs=2) as epool,
                tc.tile_pool(name="egru_ps", bufs=2, space="PSUM") as eps,
                tc.tile_pool(name="egru_st", bufs=1) as est,
            ):
                for st in range(cfg.n_ste):
                    ex = epool.tile([128, T * 256], F32, tag="ex")
                    nc.sync.dma_start(
                        out=ex[:],
                        in_=din["ed_gru"][st].rearrange("p t c -> p (t c)"))
                    hT = est.tile([128, 256], F32, tag="ehT")
                    for t in range(T):
                        xx = ex[:, t * 256:(t + 1) * 256]
                        prz = eps.tile([128, 512], F32, tag="eprz")
                        pn = eps.tile([128, 512], F32, tag="epn")
                        for gate, col, w0, b0 in (("r", 0, 0, 0),
                                                  ("z", 256, 256, 128)):
                            o = prz[:, col:col + 256]
                            steps = [(r(eKs[:, w0:w0 + 128]), r(xx))]
                            if t > 0:
                                steps.append((r(eKs[:, w0 + 128:w0 + 256]),
                                              r(hT[:])))
                            if meta["e_bias_nz"]:
                                steps.append((r(ebs[0:1, b0:b0 + 128]),
                                              r(ones512[:, 0:256])))
                            mm_chain(o, steps)
                        steps = [(r(eKs[:, 512:640]), r(xx))]
                        if meta["e_bias_nz"]:
                            steps.append((r(ebs[0:1, 256:384]),
                                          r(ones512[:, 0:256])))
                        mm_chain(pn[:, 0:256], steps)
                        if t > 0:
                            steps = [(r(eKs[:, 640:768]), r(hT[:]))]
                            if meta["e_bias_nz"]:
                                steps.append((r(ebs[0:1, 384:512]),
                                              r(ones512[:, 0:256])))
                            mm_chain(pn[:, 256:512], steps)
                        rz = epool.tile([128, 512], F32, tag="erz")
                        nc.scalar.activation(out=rz[:], in_=prz[:], func=Sig)
                        nsb = epool.tile([128, 256], F32, tag="ensb")
                        if t > 0:
                            m1 = epool.tile([128, 256], F32, tag="em1")
                            nc.vector.tensor_tensor(out=m1[:], in0=rz[:, 0:256],
                                                    in1=pn[:, 256:512],
                                                    op=mult)
                            nc.vector.tensor_tensor(out=m1[:], in0=m1[:],
                                                    in1=pn[:, 0:256], op=addop)
                            nc.scalar.activation(out=nsb[:], in_=m1[:],
                                                 func=Tnh)
                            d = epool.tile([128, 256], F32, tag="ed")
                            nc.gpsimd.tensor_tensor(out=d[:], in0=hT[:],
                                                    in1=nsb[:], op=sub)
                            nc.gpsimd.tensor_tensor(out=d[:],
                                                    in0=rz[:, 256:512],
                                                    in1=d[:], op=mult)
                            nc.vector.tensor_tensor(out=hT[:], in0=nsb[:],
                                                    in1=d[:], op=addop)
                        else:
                            nc.scalar.activation(out=nsb[:], in_=pn[:, 0:256],
                                                 func=Tnh)
                            zn = epool.tile([128, 256], F32, tag="ed")
                            nc.gpsimd.tensor_tensor(out=zn[:],
                                                    in0=rz[:, 256:512],
                                                    in1=nsb[:], op=mult)
                            nc.vector.tensor_tensor(out=hT[:], in0=nsb[:],
                                                    in1=zn[:], op=sub)
                    # decode
                    pdec = eps.tile([128, 256], F32, tag="pdec")
                    steps = [(r(sb["BDf"][:]), r(hT[:]))]
                    if meta["bf_nz"]:
                        steps.append((r(sb["bf_p"][:]), r(ones512[:, 0:256])))
                    mm_chain(pdec[:], steps)
                    dsb0 = epool.tile([128, 256], F32, tag="dsb0")
                    nc.vector.tensor_copy(out=dsb0[:], in_=pdec[:])
                    ev = e_rec[st * 2048:(st + 1) * 2048, :].rearrange(
                        "(g h c) f -> h c g f", g=8, h=2, c=128)
                    for hh in range(2):
                        ptr = eps.tile([128, 128], F32, tag="ptr")
                        nc.tensor.transpose(
                            out=r(ptr[:]),
                            in_=r(dsb0[:, hh * 128:(hh + 1) * 128]),
                            identity=r(ident128[:]))
                        dsb1 = epool.tile([128, 128], F32, tag="dsb1")
                        nc.vector.tensor_copy(out=dsb1[:], in_=ptr[:])
                        nc.sync.dma_start(
                            out=ev[hh],
                            in_=dsb1[:].rearrange("c (g f) -> c g f", f=ED))

            # ============ Phases 3/4: GCN layers ============
            def gcn_layer(layer, tab, out_cb):
                D = IN if layer == 1 else HID
                oD = HID if layer == 1 else IN
                Wd = sb["W1"] if layer == 1 else sb["W2"]
                bnz = meta["b1_nz"] if layer == 1 else meta["b2_nz"]
                bb = sb["b1"] if layer == 1 else sb["b2"]
                with (
                    tc.tile_pool(name=f"g{layer}", bufs=2) as gp,
                    tc.tile_pool(name=f"g{layer}ps", bufs=2,
                                 space="PSUM") as gps,
                    tc.tile_pool(name=f"g{layer}msg", bufs=2) as gmsg,
                    tc.tile_pool(name=f"g{layer}idx", bufs=1) as gidx,
                ):
                    idxLs = gidx.tile([128, ntL * 8], I16)
                    nc.sync.dma_start(out=idxLs[:], in_=din["idxL"])
                    idxHs = None
                    if ntH:
                        idxHs = gidx.tile([128, ntH * 8], I16)
                        nc.sync.dma_start(out=idxHs[:], in_=din["idxH"])
                    for (j0, j1, mL, mH, qs) in groups:
                        nmL, nmH = mL[1] - mL[0], mH[1] - mH[0]
                        mtile = gmsg.tile([128, max(nmL + nmH, 1), 128],
                                          BF16, tag="mtile")
                        if nmL:
                            nc.gpsimd.dma_gather(
                                out_ap=mtile[:, 0:nmL, :],
                                in_ap=tab[0:min(HALF, cfg.Npad), :],
                                idxs_ap=idxLs[:, mL[0] * 8:mL[1] * 8],
                                num_idxs=nmL * 128, num_idxs_reg=nmL * 128,
                                elem_size=128, single_packet=False)
                        if nmH:
                            nc.gpsimd.dma_gather(
                                out_ap=mtile[:, nmL:nmL + nmH, :],
                                in_ap=tab[HALF:cfg.Npad, :],
                                idxs_ap=idxHs[:, (mH[0] - ntL) * 8:
                                              (mH[1] - ntL) * 8],
                                num_idxs=nmH * 128, num_idxs_reg=nmH * 128,
                                elem_size=128, single_packet=False)
                        for j in range(j0, j1):
                            jqs = [q for q in qs if sched[q][0] == j]
                            agg = gps.tile([128, D], F32, tag="agg")
                            if not jqs:
                                nc.vector.memset(agg[:], 0.0)
                            for ii, q in enumerate(jqs):
                                _, m = sched[q]
                                mi = (m - mL[0] if m < ntL
                                      else nmL + m - mH[0])
                                S = gp.tile([128, 128], BF16, tag="S")
                                nc.vector.tensor_scalar(
                                    out=S[:], in0=iota_bf[:],
                                    scalar1=sb["offs_T"][:, q:q + 1],
                                    scalar2=None, op0=iseq)
                                nc.tensor.matmul(
                                    out=agg[:], lhsT=S[:],
                                    rhs=mtile[:, mi, 0:D],
                                    start=(ii == 0),
                                    stop=(ii == len(jqs) - 1))
                            own = (x_own[:, j * IN:(j + 1) * IN]
                                   if layer == 1
                                   else h_own[:, j * HID:(j + 1) * HID])
                            qd = gp.tile([128, D], F32, tag="qd")
                            nc.vector.tensor_scalar(
                                out=qd[:], in0=own,
                                scalar1=dinv2[:, j:j + 1], scalar2=None,
                                op0=mult)
                            pre = gp.tile([128, D], F32, tag="pre")
                            nc.vector.scalar_tensor_tensor(
                                out=pre[:], in0=agg[:],
                                scalar=dinv[:, j:j + 1], in1=qd[:],
                                op0=mult, op1=addop)
                            pT = gps.tile([128, 128], F32, tag="pT")
                            nc.tensor.transpose(out=r(pT[0:D, :]),
                                                in_=r(pre[:]),
                                                identity=r(ident128[:]))
                            pT_sb = gp.tile([D, 128], F32, tag="pTsb")
                            nc.vector.tensor_copy(out=pT_sb[:],
                                                  in_=pT[0:D, :])
                            dd = gps.tile([128, oD], F32, tag="dd")
                            steps = [(r(pT_sb[:]), r(Wd[:]))]
                            if bnz:
                                # bias varies along the free dim: ones column
                                steps.append((r(ones512[0:1, 0:128]),
                                              r(bb[:])))
                            mm_chain(dd[:], steps)
                            out_cb(j, dd, gp)

            def l1_out(j, dd, gp):
                nc.scalar.activation(out=h_own[:, j * HID:(j + 1) * HID],
                                     in_=dd[:], func=Relu)
                hs = gp.tile([128, 128], BF16, tag="hs")
                nc.vector.tensor_scalar(
                    out=hs[:], in0=h_own[:, j * HID:(j + 1) * HID],
                    scalar1=dinv[:, j:j + 1], scalar2=None, op0=mult)
                nc.sync.dma_start(out=htab_own[j * 128:(j + 1) * 128, :],
                                  in_=hs[:])

            def l2_out(j, dd, gp):
                xo = gp.tile([128, IN], F32, tag="xo")
                nc.vector.tensor_copy(out=xo[:], in_=dd[:])
                nc.sync.dma_start(out=x_rec[j * 128:(j + 1) * 128, :],
                                  in_=xo[:])

            gcn_layer(1, xtab, l1_out)
            nc.gpsimd.collective_compute(
                "AllGather", mybir.AluOpType.bypass, replica_groups=RG,
                ins=[htab_own[:, :]], outs=[htab[:, :]])
            gcn_layer(2, htab, l2_out)

    nc.compile()
    return nc


# ----------------------------------------------------------------------------
# entry points
# ----------------------------------------------------------------------------

def run_prepared(cfg, inputs, use_sim=False, trace=False):
    per_core, meta = prep_inputs(cfg, inputs)
    nc = build_program(cfg, meta)
    if use_sim:
        from concourse.bass_interp import MultiCoreSim
        sim = MultiCoreSim(nc, num_cores=cfg.C, num_workers=1)
        for k, core in sim.cores.items():
            for name, arr in per_core[k].items():
                core.tensor(name)[:] = arr
        sim.simulate(check_with_hw=False)
        res_list = [{"x_rec": np.asarray(sim.cores[k].tensor("x_rec")),
                     "e_rec": np.asarray(sim.cores[k].tensor("e_rec"))}
                    for k in range(cfg.C)]
        res = None
    else:
        res = bass_utils.run_bass_kernel_spmd(
            nc, per_core, core_ids=list(range(cfg.C)), trace=trace)
        res_list = res.results
    xs = [res_list[k]["x_rec"][:cfg.n_per] for k in range(cfg.C)]
    es = [res_list[k]["e_rec"][:cfg.e_per] for k in range(cfg.C)]
    return np.concatenate(xs, 0), np.concatenate(es, 0), res


def kernel(**inputs):
    cfg = Cfg(N=50000, E=800000, T=8, IN_DIM=64, EDGE_DIM=16, HID=128,
              n_cores=8)
    x, e, _ = run_prepared(cfg, inputs)
    return x, e


# revision 13
# speedup vs baseline: 1.3079x; 1.3079x over previous
"""DHGNN baseline kernel for 8 Trainium2 NeuronCores.

Sharding (graph/data parallel):
 - node GRU: each core encodes its own node chunk (50000/8, padded to 7168).
 - edge GRU + decoder: each core processes a contiguous 100000-edge chunk.
 - GCN: edges assigned to the core owning their dst node; per-edge source
   features come from a replicated (AllGather'd) bf16 node table via
   dma_gather; the segment-sum over dst uses one-hot selection matmuls over
   dst-sorted edges accumulating in PSUM.
The host only shards, permutes memory layouts, and builds integer index
metadata; all floating-point math runs on device.
"""

import math
import numpy as np

import concourse.bacc as bacc
import concourse.bass as bass
import concourse.mybir as mybir
import concourse.tile as tile
from concourse import bass_utils
from concourse.masks import make_identity

F32 = mybir.dt.float32
F32R = mybir.dt.float32r
BF16 = mybir.dt.bfloat16
I16 = mybir.dt.int16
I32 = mybir.dt.int32

HALF = 32768          # int16 split point for gather indices


class Cfg:
    def __init__(self, N, E, T, IN_DIM, EDGE_DIM, HID, n_cores):
        self.N, self.E, self.T = N, E, T
        self.IN, self.ED, self.HID = IN_DIM, EDGE_DIM, HID
        self.C = n_cores
        assert N % n_cores == 0 and E % n_cores == 0
        self.n_per = N // n_cores
        self.node_st = 512
        self.n_stn = math.ceil(self.n_per / self.node_st)
        self.n_pad = self.n_stn * self.node_st
        self.nt128 = self.n_pad // 128
        self.Npad = self.n_pad * n_cores
        self.e_per = E // n_cores
        self.edge_st = 2048
        self.n_ste = math.ceil(self.e_per / self.edge_st)
        self.e_pad = self.n_ste * self.edge_st
        self.jg = 4                      # node tiles per GCN gather group


def _kron8(W):
    return np.kron(np.eye(8, dtype=np.float32), np.ascontiguousarray(W.T))


# ----------------------------------------------------------------------------
# host-side prep
# ----------------------------------------------------------------------------

def prep_inputs(cfg, inputs):
    N, E, T, C = cfg.N, cfg.E, cfg.T, cfg.C
    IN, ED = cfg.IN, cfg.ED
    node_seq = np.asarray(inputs["node_seq"], np.float32)
    edge_seq = np.asarray(inputs["edge_seq"], np.float32)
    edge_index = np.asarray(inputs["edge_index"])
    src = edge_index[0].astype(np.int64)
    dst = edge_index[1].astype(np.int64)
    deg = (np.bincount(dst, minlength=N) + 1).astype(np.float32)

    per_core = [dict() for _ in range(C)]
    meta = {}

    # ---- node GRU data [n_stn, IN, T, node_st] ----
    for k in range(C):
        pad = np.zeros((cfg.n_pad, T, IN), np.float32)
        pad[:cfg.n_per] = node_seq[k * cfg.n_per:(k + 1) * cfg.n_per]
        a = pad.reshape(cfg.n_stn, cfg.node_st, T, IN).transpose(0, 3, 2, 1)
        per_core[k]["nd_gru"] = np.ascontiguousarray(a)
        d = np.ones(cfg.n_pad, np.float32)
        d[:cfg.n_per] = deg[k * cfg.n_per:(k + 1) * cfg.n_per]
        per_core[k]["deg_T"] = np.ascontiguousarray(
            d.reshape(cfg.nt128, 128).T)

    # ---- edge GRU data [n_ste, 8*ED, T, 256] ----
    for k in range(C):
        pad = np.zeros((cfg.e_pad, T, ED), np.float32)
        pad[:cfg.e_per] = edge_seq[k * cfg.e_per:(k + 1) * cfg.e_per]
        a = pad.reshape(cfg.n_ste, 8, 256, T, ED).transpose(0, 1, 4, 3, 2)
        per_core[k]["ed_gru"] = np.ascontiguousarray(
            a.reshape(cfg.n_ste, 8 * ED, T, 256))

    # ---- GCN: per-core dst-owned edges, dst-sorted, L/H split on padded src
    owner = dst // cfg.n_per
    pg = (src // cfg.n_per) * cfg.n_pad + (src % cfg.n_per)
    core_edges = []
    for k in range(C):
        sel = np.nonzero(owner == k)[0]
        dl = dst[sel] - k * cfg.n_per
        sg = pg[sel]
        hi = (sg >= HALF).astype(np.int64)
        order = np.lexsort((dl, hi))
        core_edges.append((dl[order], sg[order], int((hi == 0).sum())))
    ntL = max(math.ceil(m[2] / 128) for m in core_edges)
    ntH = max(math.ceil((len(m[0]) - m[2]) / 128) for m in core_edges)
    NT = ntL + ntH
    meta["ntL"], meta["ntH"], meta["NT"] = ntL, ntH, NT

    # per-core padded tile arrays + union pair schedule
    all_dl = []
    for k in range(C):
        dl, sg, nL = core_edges[k]
        nH = len(dl) - nL
        dla = np.full(NT * 128, -1, np.int64)
        sga = np.zeros(NT * 128, np.int64)
        dla[:nL], sga[:nL] = dl[:nL], sg[:nL]
        dla[ntL * 128:ntL * 128 + nH] = dl[nL:]
        sga[ntL * 128:ntL * 128 + nH] = sg[nL:]
        gl = np.where(dla[:ntL * 128] >= 0, sga[:ntL * 128], 0)
        gh = np.where(dla[ntL * 128:] >= 0, sga[ntL * 128:] - HALF, 0)
        per_core[k]["idxL"] = np.ascontiguousarray(
            np.tile(gl.reshape(-1, 16).T.astype(np.int16), (8, 1)))
        ihw = gh.reshape(-1, 16).T.astype(np.int16) if gh.size else \
            np.zeros((16, 8), np.int16)
        per_core[k]["idxH"] = np.ascontiguousarray(np.tile(ihw, (8, 1)))
        all_dl.append(dla)

    pair_keys = set()
    core_pairs = []
    for k in range(C):
        dla = all_dl[k]
        pd = {}
        for m in range(NT):
            d128 = dla[m * 128:(m + 1) * 128]
            real = d128 >= 0
            if not real.any():
                continue
            for t in range(int(d128[real].min() // 128),
                           int(d128[real].max() // 128) + 1):
                offs = np.where(real, d128 - t * 128, -4096).astype(np.float32)
                pd[(t, m)] = offs
                pair_keys.add((t, m))
        core_pairs.append(pd)
    sched = sorted(pair_keys)                      # grouped by node tile t
    NPs = max(len(sched), 1)
    meta["sched"] = sched
    meta["NPs"] = NPs
    for k in range(C):
        offs_T = np.full((128, NPs), -4096.0, np.float32)
        for q, key in enumerate(sched):
            if key in core_pairs[k]:
                offs_T[:, q] = core_pairs[k][key]
        per_core[k]["offs_T"] = np.ascontiguousarray(offs_T)

    # gather groups: node tiles grouped by cfg.jg; m-ranges per section
    groups = []
    n_groups = math.ceil(cfg.nt128 / cfg.jg)
    for gidx in range(n_groups):
        j0, j1 = gidx * cfg.jg, min((gidx + 1) * cfg.jg, cfg.nt128)
        qs = [q for q, (t, m) in enumerate(sched) if j0 <= t < j1]
        msL = [sched[q][1] for q in qs if sched[q][1] < ntL]
        msH = [sched[q][1] for q in qs if sched[q][1] >= ntL]
        mL = (min(msL), max(msL) + 1) if msL else (0, 0)
        mH = (min(msH), max(msH) + 1) if msH else (ntL, ntL)
        groups.append((j0, j1, mL, mH, qs))
    meta["groups"] = groups

    # ---- weights ----
    w = {n: np.asarray(inputs[n], np.float32) for n in
         ["W_ih_n", "W_hh_n", "b_ih_n", "b_hh_n",
          "W_ih_e", "W_hh_e", "b_ih_e", "b_hh_e",
          "W1", "b1", "W2", "b2", "Wf", "bf"]}
    H = ED
    eK = np.concatenate([
        _kron8(w["W_ih_e"][0:H]), _kron8(w["W_hh_e"][0:H]),
        _kron8(w["W_ih_e"][H:2 * H]), _kron8(w["W_hh_e"][H:2 * H]),
        _kron8(w["W_ih_e"][2 * H:3 * H]), _kron8(w["W_hh_e"][2 * H:3 * H]),
    ], axis=1)                                     # [128, 768]
    ebias = np.zeros((1, 512), np.float32)
    for g in range(8):
        ebias[0, 0 + g * H:0 + (g + 1) * H] = (w["b_ih_e"][0:H]
                                               + w["b_hh_e"][0:H])
        ebias[0, 128 + g * H:128 + (g + 1) * H] = (w["b_ih_e"][H:2 * H]
                                                   + w["b_hh_e"][H:2 * H])
        ebias[0, 256 + g * H:256 + (g + 1) * H] = w["b_ih_e"][2 * H:3 * H]
        ebias[0, 384 + g * H:384 + (g + 1) * H] = w["b_hh_e"][2 * H:3 * H]
    D = IN
    nK = np.concatenate([
        w["W_ih_n"][0:D].T, w["W_hh_n"][0:D].T,
        w["W_ih_n"][D:2 * D].T, w["W_hh_n"][D:2 * D].T,
        w["W_ih_n"][2 * D:3 * D].T, w["W_hh_n"][2 * D:3 * D].T,
    ], axis=1)                                     # [64, 384]
    nbias = np.zeros((1, 256), np.float32)
    nbias[0, 0:D] = w["b_ih_n"][0:D] + w["b_hh_n"][0:D]
    nbias[0, 64:64 + D] = w["b_ih_n"][D:2 * D] + w["b_hh_n"][D:2 * D]
    nbias[0, 128:128 + D] = w["b_ih_n"][2 * D:3 * D]
    nbias[0, 192:192 + D] = w["b_hh_n"][2 * D:3 * D]
    BDf = _kron8(w["Wf"])                          # [128, 128]
    bfp = np.zeros((1, 128), np.float32)
    for g in range(8):
        bfp[0, g * H:(g + 1) * H] = w["bf"]
    wk = dict(eK=np.ascontiguousarray(eK.astype(np.float32)),
              e_bias=ebias, nK=np.ascontiguousarray(nK.astype(np.float32)),
              n_bias=nbias, BDf=np.ascontiguousarray(BDf.astype(np.float32)),
              bf_p=bfp,
              W1=np.ascontiguousarray(w["W1"]),
              W2=np.ascontiguousarray(w["W2"]),
              b1=w["b1"][None, :].copy(), b2=w["b2"][None, :].copy())
    meta["e_bias_nz"] = bool(np.any(ebias != 0))
    meta["n_bias_nz"] = bool(np.any(nbias != 0))
    meta["bf_nz"] = bool(np.any(w["bf"] != 0))
    meta["b1_nz"] = bool(np.any(w["b1"] != 0))
    meta["b2_nz"] = bool(np.any(w["b2"] != 0))
    for k in range(C):
        per_core[k].update(wk)
    return per_core, meta


# ----------------------------------------------------------------------------
# device program
# ----------------------------------------------------------------------------

def build_program(cfg, meta, phases=("node", "edge", "gcn")):
    nc = bacc.Bacc("TRN2", num_devices=cfg.C, debug=False)
    T, IN, ED, HID = cfg.T, cfg.IN, cfg.ED, cfg.HID
    nt128 = cfg.nt128
    ntL, ntH, NT = meta["ntL"], meta["ntH"], meta["NT"]
    NPs, sched, groups = meta["NPs"], meta["sched"], meta["groups"]
    Sig = mybir.ActivationFunctionType.Sigmoid
    Tnh = mybir.ActivationFunctionType.Tanh
    Relu = mybir.ActivationFunctionType.Relu
    mult, addop = mybir.AluOpType.mult, mybir.AluOpType.add
    sub, iseq = mybir.AluOpType.subtract, mybir.AluOpType.is_equal

    USE_F32R = False

    def r(ap):
        return ap.bitcast(F32R) if USE_F32R else ap

    def mm_chain(out, steps):
        """steps: list of (lhsT, rhs); emits an accumulation group."""
        for i, (lt, rh) in enumerate(steps):
            nc.tensor.matmul(out=out, lhsT=lt, rhs=rh, start=(i == 0),
                             stop=(i == len(steps) - 1),
                             skip_group_check=(len(steps) > 1))

    din = {}
    for name, shape, dt in [
        ("nd_gru", [cfg.n_stn, IN, T, cfg.node_st], F32),
        ("ed_gru", [cfg.n_ste, 8 * ED, T, 256], F32),
        ("deg_T", [128, nt128], F32),
        ("idxL", [128, ntL * 8], I16),
        ("idxH", [128, max(ntH, 1) * 8], I16),
        ("offs_T", [128, NPs], F32),
        ("eK", [128, 768], F32), ("e_bias", [1, 512], F32),
        ("nK", [64, 384], F32), ("n_bias", [1, 256], F32),
        ("BDf", [128, 128], F32), ("bf_p", [1, 128], F32),
        ("W1", [IN, HID], F32), ("W2", [HID, IN], F32),
        ("b1", [1, HID], F32), ("b2", [1, IN], F32),
    ]:
        din[name] = nc.dram_tensor(name, shape, dt, kind="ExternalInput").ap()

    x_rec = nc.dram_tensor("x_rec", [cfg.n_pad, IN], F32,
                           kind="ExternalOutput").ap()
    e_rec = nc.dram_tensor("e_rec", [cfg.e_pad, ED], F32,
                           kind="ExternalOutput").ap()
    RG = [[i for i in range(cfg.C)]]

    with tile.TileContext(nc) as tc:
        with (
            tc.tile_pool(name="const", bufs=1) as cpool,
            tc.tile_pool(name="persist", bufs=1) as ppool,
            tc.tile_pool(name="dram", bufs=1, space="DRAM") as dpool,
        ):
            sb = {}
            for name in ["eK", "e_bias", "nK", "n_bias", "BDf", "bf_p",
                         "W1", "W2", "b1", "b2", "deg_T", "offs_T"]:
                t_ = cpool.tile(din[name].shape, F32, name="c_" + name)
                nc.sync.dma_start(out=t_[:], in_=din[name])
                sb[name] = t_
            ones512 = cpool.tile([1, 512], F32)
            nc.gpsimd.memset(ones512[:], 1.0)
            iota_i32 = cpool.tile([128, 128], I32)
            nc.gpsimd.iota(iota_i32[:], pattern=[[1, 128]], base=0,
                           channel_multiplier=0)
            iota_bf = cpool.tile([128, 128], BF16)
            nc.vector.tensor_copy(out=iota_bf[:], in_=iota_i32[:])
            ident64 = cpool.tile([64, 64], F32)
            make_identity(nc, ident64[:])
            ident128 = cpool.tile([128, 128], F32)
            make_identity(nc, ident128[:])

            # dinv = 1/sqrt(deg), with one Newton step to clean up ACT sqrt:
            # y = sqrt(x); y1 = 0.5*(y + x/y)
            rec = ppool.tile([128, nt128], F32)
            nc.vector.reciprocal(out=rec[:], in_=sb["deg_T"][:])   # x = 1/deg
            y0 = ppool.tile([128, nt128], F32)
            nc.scalar.sqrt(out=y0[:], in_=rec[:])
            yr = ppool.tile([128, nt128], F32)
            nc.vector.reciprocal(out=yr[:], in_=y0[:])
            dinv = ppool.tile([128, nt128], F32)
            nc.vector.tensor_tensor(out=dinv[:], in0=rec[:], in1=yr[:],
                                    op=mult)                       # x/y
            nc.vector.tensor_tensor(out=dinv[:], in0=dinv[:], in1=y0[:],
                                    op=addop)
            nc.vector.tensor_scalar(out=dinv[:], in0=dinv[:], scalar1=0.5,
                                    scalar2=None, op0=mult)
            dinv2 = ppool.tile([128, nt128], F32)
            nc.vector.tensor_tensor(out=dinv2[:], in0=dinv[:], in1=dinv[:],
                                    op=mult)

            x_own = ppool.tile([128, nt128 * IN], F32)
            h_own = ppool.tile([128, nt128 * HID], F32)

            xtab_own = dpool.tile([cfg.n_pad, 128], BF16)
            xtab = dpool.tile([cfg.Npad, 128], BF16, addr_space="Shared")
            htab_own = dpool.tile([cfg.n_pad, 128], BF16)
            htab = dpool.tile([cfg.Npad, 128], BF16, addr_space="Shared")

            nKs, ebs, nbs = sb["nK"], sb["e_bias"], sb["n_bias"]

            do_node = "node" in phases
            do_edge = "edge" in phases
            do_gcn = "gcn" in phases
            # ============ Phase 1: node GRU ============
            # state/gates feature-major [64, 512] per 512-node supertile
            with (
                tc.tile_pool(name="ngru", bufs=2) as npool,
                tc.tile_pool(name="ngru_ps", bufs=1, space="PSUM") as nps,
                tc.tile_pool(name="ngru_ps2", bufs=2, space="PSUM") as nps2,
                tc.tile_pool(name="ngru_st", bufs=1) as nst,
            ):
                for st in range(cfg.n_stn if do_node else 0):
                    xt_all = npool.tile([64, T * 512], F32, tag="xt")
                    nc.sync.dma_start(
                        out=xt_all[:],
                        in_=din["nd_gru"][st].rearrange("f t c -> f (t c)"))
                    hT = nst.tile([64, 512], F32, tag="hT")
                    for t in range(T):
                        xx = xt_all[:, t * 512:(t + 1) * 512]
                        ps_rz = nps.tile([64, 1024], F32, tag="psrz")
                        ps_gin = nps.tile([64, 512], F32, tag="psgin")
                        for gate, col, w0, b0 in (("r", 0, 0, 0),
                                                  ("z", 512, 128, 64)):
                            o = ps_rz[:, col:col + 512]
                            steps = [(r(nKs[:, w0:w0 + 64]), r(xx))]
                            if t > 0:
                                steps.append((r(nKs[:, w0 + 64:w0 + 128]),
                                              r(hT[:])))
                            if meta["n_bias_nz"]:
                                steps.append((r(nbs[0:1, b0:b0 + 64]),
                                              r(ones512[:, 0:512])))
                            mm_chain(o, steps)
                        steps = [(r(nKs[:, 256:320]), r(xx))]
                        if meta["n_bias_nz"]:
                            steps.append((r(nbs[0:1, 128:192]),
                                          r(ones512[:, 0:512])))
                        mm_chain(ps_gin[:], steps)
                        if t > 0:
                            ps_ghn = nps.tile([64, 512], F32, tag="psghn")
                            steps = [(r(nKs[:, 320:384]), r(hT[:]))]
                            if meta["n_bias_nz"]:
                                steps.append((r(nbs[0:1, 192:256]),
                                              r(ones512[:, 0:512])))
                            mm_chain(ps_ghn[:], steps)
                        rz = npool.tile([64, 1024], F32, tag="rz")
                        nc.scalar.activation(out=rz[:], in_=ps_rz[:], func=Sig)
                        n_sb = npool.tile([64, 512], F32, tag="nsb")
                        if t > 0:
                            m1 = npool.tile([64, 512], F32, tag="m1")
                            nc.vector.tensor_tensor(out=m1[:],
                                                    in0=rz[:, 0:512],
                                                    in1=ps_ghn[:], op=mult)
                            nc.vector.tensor_tensor(out=m1[:], in0=m1[:],
                                                    in1=ps_gin[:], op=addop)
                            nc.scalar.activation(out=n_sb[:], in_=m1[:],
                                                 func=Tnh)
                            d = npool.tile([64, 512], F32, tag="d")
                            nc.gpsimd.tensor_tensor(out=d[:], in0=hT[:],
                                                    in1=n_sb[:], op=sub)
                            nc.gpsimd.tensor_tensor(out=d[:],
                                                    in0=rz[:, 512:1024],
                                                    in1=d[:], op=mult)
                            nc.vector.tensor_tensor(out=hT[:], in0=n_sb[:],
                                                    in1=d[:], op=addop)
                        else:
                            nc.scalar.activation(out=n_sb[:], in_=ps_gin[:],
                                                 func=Tnh)
                            zn = npool.tile([64, 512], F32, tag="d")
                            nc.gpsimd.tensor_tensor(out=zn[:],
                                                    in0=rz[:, 512:1024],
                                                    in1=n_sb[:], op=mult)
                            nc.vector.tensor_tensor(out=hT[:], in0=n_sb[:],
                                                    in1=zn[:], op=sub)
                    # x_t node-major + bf16 table rows (4 tiles of 128 nodes)
                    for j8 in range(4):
                        j = st * 4 + j8
                        pxt = nps2.tile([128, 64], F32, tag="pxt")
                        nc.tensor.transpose(
                            out=r(pxt[:]),
                            in_=r(hT[:, j8 * 128:(j8 + 1) * 128]),
                            identity=r(ident64[:]))
                        nc.vector.tensor_copy(
                            out=x_own[:, j * IN:(j + 1) * IN], in_=pxt[:])
                        xs = npool.tile([128, 128], BF16, tag="xs")
                        nc.gpsimd.memset(xs[:], 0.0)
                        nc.vector.tensor_scalar(
                            out=xs[:, 0:IN],
                            in0=x_own[:, j * IN:(j + 1) * IN],
                            scalar1=dinv[:, j:j + 1], scalar2=None, op0=mult)
                        nc.sync.dma_start(
                            out=xtab_own[j * 128:(j + 1) * 128, :], in_=xs[:])

            if do_node and do_gcn:
                nc.gpsimd.collective_compute(
                    "AllGather", mybir.AluOpType.bypass, replica_groups=RG,
                    ins=[xtab_own[:, :]], outs=[xtab[:, :]])

            # ============ Phase 2: edge GRU + decode ============
            eKs = sb["eK"]
            with (
                tc.tile_pool(name="egru", bufs=2) as epool,
                tc.tile_pool(name="egru_ps", bufs=2, space="PSUM") as eps,
                tc.tile_pool(name="egru_st", bufs=1) as est,
            ):
                for st in range(cfg.n_ste if do_edge else 0):
                    ex = epool.tile([128, T * 256], F32, tag="ex")
                    nc.sync.dma_start(
                        out=ex[:],
                        in_=din["ed_gru"][st].rearrange("p t c -> p (t c)"))
                    hT = est.tile([128, 256], F32, tag="ehT")
                    for t in range(T):
                        xx = ex[:, t * 256:(t + 1) * 256]
                        prz = eps.tile([128, 512], F32, tag="eprz")
                        pn = eps.tile([128, 512], F32, tag="epn")
                        for gate, col, w0, b0 in (("r", 0, 0, 0),
                                                  ("z", 256, 256, 128)):
                            o = prz[:, col:col + 256]
                            steps = [(r(eKs[:, w0:w0 + 128]), r(xx))]
                            if t > 0:
                                steps.append((r(eKs[:, w0 + 128:w0 + 256]),
                                              r(hT[:])))
                            if meta["e_bias_nz"]:
                                steps.append((r(ebs[0:1, b0:b0 + 128]),
                                              r(ones512[:, 0:256])))
                            mm_chain(o, steps)
                        steps = [(r(eKs[:, 512:640]), r(xx))]
                        if meta["e_bias_nz"]:
                            steps.append((r(ebs[0:1, 256:384]),
                                          r(ones512[:, 0:256])))
                        mm_chain(pn[:, 0:256], steps)
                        if t > 0:
                            steps = [(r(eKs[:, 640:768]), r(hT[:]))]
                            if meta["e_bias_nz"]:
                                steps.append((r(ebs[0:1, 384:512]),
                                              r(ones512[:, 0:256])))
                            mm_chain(pn[:, 256:512], steps)
                        rz = epool.tile([128, 512], F32, tag="erz")
                        nc.scalar.activation(out=rz[:], in_=prz[:], func=Sig)
                        nsb = epool.tile([128, 256], F32, tag="ensb")
                        if t > 0:
                            m1 = epool.tile([128, 256], F32, tag="em1")
                            nc.vector.tensor_tensor(out=m1[:], in0=rz[:, 0:256],
                                                    in1=pn[:, 256:512],
                                                    op=mult)
                            nc.vector.tensor_tensor(out=m1[:], in0=m1[:],
                                                    in1=pn[:, 0:256], op=addop)
                            nc.scalar.activation(out=nsb[:], in_=m1[:],
                                                 func=Tnh)
                            d = epool.tile([128, 256], F32, tag="ed")
                            nc.gpsimd.tensor_tensor(out=d[:], in0=hT[:],
                                                    in1=nsb[:], op=sub)
                            nc.gpsimd.tensor_tensor(out=d[:],
                                                    in0=rz[:, 256:512],
                                                    in1=d[:], op=mult)
                            nc.vector.tensor_tensor(out=hT[:], in0=nsb[:],
                                                    in1=d[:], op=addop)
                        else:
                            nc.scalar.activation(out=nsb[:], in_=pn[:, 0:256],
                                                 func=Tnh)
                            zn = epool.tile([128, 256], F32, tag="ed")
                            nc.gpsimd.tensor_tensor(out=zn[:],
                                                    in0=rz[:, 256:512],
                                                    in1=nsb[:], op=mult)
                            nc.vector.tensor_tensor(out=hT[:], in0=nsb[:],
                                                    in1=zn[:], op=sub)
                    # decode
                    pdec = eps.tile([128, 256], F32, tag="pdec")
                    steps = [(r(sb["BDf"][:]), r(hT[:]))]
                    if meta["bf_nz"]:
                        steps.append((r(sb["bf_p"][:]), r(ones512[:, 0:256])))
                    mm_chain(pdec[:], steps)
                    dsb0 = epool.tile([128, 256], F32, tag="dsb0")
                    nc.vector.tensor_copy(out=dsb0[:], in_=pdec[:])
                    ev = e_rec[st * 2048:(st + 1) * 2048, :].rearrange(
                        "(g h c) f -> h c g f", g=8, h=2, c=128)
                    for hh in range(2):
                        ptr = eps.tile([128, 128], F32, tag="ptr")
                        nc.tensor.transpose(
                            out=r(ptr[:]),
                            in_=r(dsb0[:, hh * 128:(hh + 1) * 128]),
                            identity=r(ident128[:]))
                        dsb1 = epool.tile([128, 128], F32, tag="dsb1")
                        nc.vector.tensor_copy(out=dsb1[:], in_=ptr[:])
                        nc.sync.dma_start(
                            out=ev[hh],
                            in_=dsb1[:].rearrange("c (g f) -> c g f", f=ED))

            # ============ Phases 3/4: GCN layers ============
            def gcn_layer(layer, tab, out_cb):
                D = IN if layer == 1 else HID
                oD = HID if layer == 1 else IN
                Wd = sb["W1"] if layer == 1 else sb["W2"]
                bnz = meta["b1_nz"] if layer == 1 else meta["b2_nz"]
                bb = sb["b1"] if layer == 1 else sb["b2"]
                with (
                    tc.tile_pool(name=f"g{layer}", bufs=2) as gp,
                    tc.tile_pool(name=f"g{layer}ps", bufs=2,
                                 space="PSUM") as gps,
                    tc.tile_pool(name=f"g{layer}msg", bufs=2) as gmsg,
                    tc.tile_pool(name=f"g{layer}idx", bufs=1) as gidx,
                ):
                    idxLs = gidx.tile([128, ntL * 8], I16)
                    nc.sync.dma_start(out=idxLs[:], in_=din["idxL"])
                    idxHs = None
                    if ntH:
                        idxHs = gidx.tile([128, ntH * 8], I16)
                        nc.sync.dma_start(out=idxHs[:], in_=din["idxH"])
                    for (j0, j1, mL, mH, qs) in groups:
                        nmL, nmH = mL[1] - mL[0], mH[1] - mH[0]
                        mtile = gmsg.tile([128, max(nmL + nmH, 1), 128],
                                          BF16, tag="mtile")
                        if nmL:
                            nc.gpsimd.dma_gather(
                                out_ap=mtile[:, 0:nmL, :],
                                in_ap=tab[0:min(HALF, cfg.Npad), :],
                                idxs_ap=idxLs[:, mL[0] * 8:mL[1] * 8],
                                num_idxs=nmL * 128, num_idxs_reg=nmL * 128,
                                elem_size=128, single_packet=False)
                        if nmH:
                            nc.gpsimd.dma_gather(
                                out_ap=mtile[:, nmL:nmL + nmH, :],
                                in_ap=tab[HALF:cfg.Npad, :],
                                idxs_ap=idxHs[:, (mH[0] - ntL) * 8:
                                              (mH[1] - ntL) * 8],
                                num_idxs=nmH * 128, num_idxs_reg=nmH * 128,
                                elem_size=128, single_packet=False)
                        for j in range(j0, j1):
                            jqs = [q for q in qs if sched[q][0] == j]
                            agg = gps.tile([128, D], F32, tag="agg")
                            if not jqs:
                                nc.vector.memset(agg[:], 0.0)
                            for ii, q in enumerate(jqs):
                                _, m = sched[q]
                                mi = (m - mL[0] if m < ntL
                                      else nmL + m - mH[0])
                                S = gp.tile([128, 128], BF16, tag="S")
                                nc.vector.tensor_scalar(
                                    out=S[:], in0=iota_bf[:],
                                    scalar1=sb["offs_T"][:, q:q + 1],
                                    scalar2=None, op0=iseq)
                                nc.tensor.matmul(
                                    out=agg[:], lhsT=S[:],
                                    rhs=mtile[:, mi, 0:D],
                                    start=(ii == 0),
                                    stop=(ii == len(jqs) - 1))
                            own = (x_own[:, j * IN:(j + 1) * IN]
                                   if layer == 1
                                   else h_own[:, j * HID:(j + 1) * HID])
                            qd = gp.tile([128, D], F32, tag="qd")
                            nc.vector.tensor_scalar(
                                out=qd[:], in0=own,
                                scalar1=dinv2[:, j:j + 1], scalar2=None,
                                op0=mult)
                            pre = gp.tile([128, D], F32, tag="pre")
                            nc.vector.scalar_tensor_tensor(
                                out=pre[:], in0=agg[:],
                                scalar=dinv[:, j:j + 1], in1=qd[:],
                                op0=mult, op1=addop)
                            pT = gps.tile([128, 128], F32, tag="pT")
                            nc.tensor.transpose(out=r(pT[0:D, :]),
                                                in_=r(pre[:]),
                                                identity=r(ident128[:]))
                            pT_sb = gp.tile([D, 128], F32, tag="pTsb")
                            nc.vector.tensor_copy(out=pT_sb[:],
                                                  in_=pT[0:D, :])
                            dd = gps.tile([128, oD], F32, tag="dd")
                            steps = [(r(pT_sb[:]), r(Wd[:]))]
                            if bnz:
                                # bias varies along the free dim: ones column
                                steps.append((r(ones512[0:1, 0:128]),
                                              r(bb[:])))
                            mm_chain(dd[:], steps)
                            out_cb(j, dd, gp)

            def l1_out(j, dd, gp):
                nc.scalar.activation(out=h_own[:, j * HID:(j + 1) * HID],
                                     in_=dd[:], func=Relu)
                hs = gp.tile([128, 128], BF16, tag="hs")
                nc.vector.tensor_scalar(
                    out=hs[:], in0=h_own[:, j * HID:(j + 1) * HID],
                    scalar1=dinv[:, j:j + 1], scalar2=None, op0=mult)
                nc.sync.dma_start(out=htab_own[j * 128:(j + 1) * 128, :],
                                  in_=hs[:])

            def l2_out(j, dd, gp):
                xo = gp.tile([128, IN], F32, tag="xo")
                nc.vector.tensor_copy(out=xo[:], in_=dd[:])
                nc.sync.dma_start(out=x_rec[j * 128:(j + 1) * 128, :],
                                  in_=xo[:])

            if do_gcn and not do_node:
                # tables uninitialized; still exercise gathers
                pass
            if do_gcn:
                gcn_layer(1, xtab, l1_out)
                nc.gpsimd.collective_compute(
                    "AllGather", mybir.AluOpType.bypass, replica_groups=RG,
                    ins=[htab_own[:, :]], outs=[htab[:, :]])
                gcn_layer(2, htab, l2_out)
            else:
                z0 = cpool.tile([128, IN], F32)
                nc.gpsimd.memset(z0[:], 0.0)
                for j in range(nt128):
                    nc.sync.dma_start(out=x_rec[j * 128:(j + 1) * 128, :],
                                      in_=z0[:])

    nc.compile()
    return nc


# ----------------------------------------------------------------------------
# entry points
# ----------------------------------------------------------------------------

def run_prepared(cfg, inputs, use_sim=False, trace=False):
    per_core, meta = prep_inputs(cfg, inputs)
    nc = build_program(cfg, meta)
    if use_sim:
        from concourse.bass_interp import MultiCoreSim
        sim = MultiCoreSim(nc, num_cores=cfg.C, num_workers=1)
        for k, core in sim.cores.items():
            for name, arr in per_core[k].items():
                core.tensor(name)[:] = arr
        sim.simulate(check_with_hw=False)
        res_list = [{"x_rec": np.asarray(sim.cores[k].tensor("x_rec")),
                     "e_rec": np.asarray(sim.cores[k].tensor("e_rec"))}
                    for k in range(cfg.C)]
        res = None
    else:
        res = bass_utils.run_bass_kernel_spmd(
            nc, per_core, core_ids=list(range(cfg.C)), trace=trace)
        res_list = res.results
    xs = [res_list[k]["x_rec"][:cfg.n_per] for k in range(cfg.C)]
    es = [res_list[k]["e_rec"][:cfg.e_per] for k in range(cfg.C)]
    return np.concatenate(xs, 0), np.concatenate(es, 0), res


def kernel(**inputs):
    cfg = Cfg(N=50000, E=800000, T=8, IN_DIM=64, EDGE_DIM=16, HID=128,
              n_cores=8)
    x, e, _ = run_prepared(cfg, inputs)
    return x, e
